# revision 8
# baseline (speedup 1.0000x reference)
"""Trainium2 Bass kernel: differentiable Gaussian-splat renderer.

Math: image[b,h,w,c] = clip( sum_n exp(-a_n*((gx_w-px_n)^2+(gy_h-py_n)^2)) * col[n,c], 0, 1 )
with a_n = 1/(2*sigma_n^2+1e-8), sigma_n = sizes_n*2/H.

The Gaussian separates: exp(-a*(dx^2+dy^2)) = exp(-a*dx^2)*exp(-a*dy^2), so per
frame the image is a matmul over splats:
    image[h, (w,c)] = sum_n wy[n,h] * (wx[n,w]*col[n,c])

d2 is produced by a tiny K-dim polynomial matmul on the PE:
    d2[n, g] = 1*g^2 + (-2p_n)*g + p_n^2
then ONE Exp activation with per-partition scale (-a_n) gives wx|wy.

fp32r (11-bit mantissa) would destroy the d2 cancellation for small sigma, so
both polynomial operands are split hi/lo into fp32r pairs (error-free products,
~2^-22 effective precision) -- K grows 6->18 which costs nothing on the PE.

Sharding: data-parallel over B: 16 frames -> 8 cores x 2 frames.
"""

import numpy as np

H = 224
W = 224
NPTS = 381
CH = 3
B = 16
NCORES = 8
BPC = B // NCORES   # frames per core
NCHUNK = 3          # point chunks; n = 3*p + j  (381 = 127*3)
KC = NPTS // NCHUNK  # 127 points per chunk (contraction partitions)
HC = 112            # h-chunk size (224 = 2*112 output partitions)
NH = 336            # moving-dim half (672 = 2*336); >=256 keeps f32r at full rate


def _round_f32r(x):
    """Round float32 array to fp32r (keep top 11 mantissa bits, round-nearest)."""
    u = np.ascontiguousarray(x, dtype=np.float32).view(np.uint32)
    low = u & 0xFFF
    up = (low > 0x800) | ((low == 0x800) & (((u >> 12) & 1) == 1))
    r = (u & ~np.uint32(0xFFF)) + np.where(up, np.uint32(0x1000), np.uint32(0))
    return r.view(np.float32)


def _np_grid18():
    """[18, 448] fp32r-valued rows; row 3r+t pairs with lhsT col t in {hi,hi,lo}.
    r in 0..5 = (g^2, g, 1) for x-half cols [0:224], same for y-half [224:448]."""
    g = -1.0 + (2.0 / (W - 1)) * np.arange(W, dtype=np.float64)
    R = np.zeros((6, 2 * W), dtype=np.float64)
    R[0, 0:W] = g * g
    R[1, 0:W] = g
    R[2, 0:W] = 1.0
    R[3, W:] = g * g
    R[4, W:] = g
    R[5, W:] = 1.0
    out = np.zeros((18, 2 * W), dtype=np.float32)
    for r in range(6):
        hi = _round_f32r(R[r].astype(np.float32))
        lo = _round_f32r((R[r] - hi.astype(np.float64)).astype(np.float32))
        out[3 * r + 0] = hi   # pairs L_hi
        out[3 * r + 1] = lo   # pairs L_hi
        out[3 * r + 2] = hi   # pairs L_lo
    return out


def build_bass():
    import concourse.bacc as bacc
    import concourse.tile as tile
    from concourse import mybir

    f32 = mybir.dt.float32
    f32r = mybir.dt.float32r
    Act = mybir.ActivationFunctionType
    Alu = mybir.AluOpType

    nc = bacc.Bacc("TRN2", debug=False, enable_partition_id=False)

    pos_d = nc.dram_tensor("positions", [BPC, NPTS, 2], f32, kind="ExternalInput")
    col_d = nc.dram_tensor("colors", [BPC, NPTS, CH], f32, kind="ExternalInput")
    siz_d = nc.dram_tensor("sizes", [BPC, NPTS], f32, kind="ExternalInput")
    grid_d = nc.dram_tensor("grid18", [18, 2 * W], f32r, kind="ExternalInput")
    id_d = nc.dram_tensor("ident", [128, 128], f32r, kind="ExternalInput")
    img_d = nc.dram_tensor("image", [BPC, H, W, CH], f32, kind="ExternalOutput")

    with tile.TileContext(nc) as tc:
        with (
            tc.tile_pool(name="const", bufs=1) as constp,
            tc.tile_pool(name="inp", bufs=2) as inp,
            tc.tile_pool(name="small", bufs=2) as small,
            tc.tile_pool(name="big", bufs=2) as big,
            tc.tile_pool(name="outp", bufs=4) as outp,
            tc.tile_pool(name="ps_tr", bufs=1, space="PSUM") as ps_tr,
            tc.tile_pool(name="ps_z", bufs=2, space="PSUM") as ps_z,
            tc.tile_pool(name="ps_out", bufs=4, space="PSUM") as ps_out,
        ):
            grid18 = constp.tile([18, 2 * W], f32r)
            nc.sync.dma_start(out=grid18, in_=grid_d[:])
            ident = constp.tile([128, 128], f32r)
            nc.sync.dma_start(out=ident, in_=id_d[:])

            for b in range(BPC):
                # ---- inputs; interleaved chunking: point n=3p+j -> (partition p, chunk j)
                pos6 = inp.tile([128, 2 * NCHUNK], f32, tag="pos")  # cols (j,e)
                nc.sync.dma_start(
                    out=pos6[0:KC, :],
                    in_=pos_d[b].rearrange("(p j) e -> p (j e)", j=NCHUNK),
                )
                sz3 = inp.tile([128, NCHUNK], f32, tag="sz")
                nc.sync.dma_start(
                    out=sz3[0:KC, :], in_=siz_d[b].rearrange("(p j) -> p j", j=NCHUNK)
                )
                col9 = inp.tile([128, CH * NCHUNK], f32, tag="col")  # cols (j,c)
                nc.sync.dma_start(
                    out=col9[0:KC, :],
                    in_=col_d[b].rearrange("(p j) c -> p (j c)", j=NCHUNK),
                )

                # ---- per-point exp scale: an = -1/(2*sigma^2 + 1e-8)  [*, 3]
                sq = small.tile([128, NCHUNK], f32, tag="sq")
                nc.scalar.activation(
                    out=sq[0:KC, :], in_=sz3[0:KC, :], func=Act.Square, scale=2.0 / H
                )
                u = small.tile([128, NCHUNK], f32, tag="u")
                nc.vector.tensor_scalar(
                    out=u[0:KC, :], in0=sq[0:KC, :], scalar1=-2.0, scalar2=-1e-8,
                    op0=Alu.mult, op1=Alu.add,
                )
                an = small.tile([128, NCHUNK], f32, tag="an")
                nc.vector.reciprocal(out=an[0:KC, :], in_=u[0:KC, :])

                # ---- polynomial coeffs L6[p, 6j+r] = (1, -2px, px^2, 1, -2py, py^2)
                L6 = small.tile([128, 6 * NCHUNK], f32, tag="L6")
                nc.vector.memset(L6, 0.0)
                L6r = L6.rearrange("p (j r) -> p j r", r=6)
                posr = pos6.rearrange("p (j e) -> p j e", e=2)
                ones = L6r[0:KC].rearrange("p j (s r) -> p j s r", s=2)  # r in {0..2},{3..5}
                nc.vector.memset(ones[:, :, :, 0], 1.0)  # rows 1
                nc.vector.tensor_scalar(
                    out=ones[:, :, :, 1], in0=posr[0:KC], scalar1=-2.0, scalar2=None,
                    op0=Alu.mult,
                )
                nc.vector.tensor_mul(ones[:, :, :, 2], posr[0:KC], posr[0:KC])

                # ---- fp32r hi/lo split: Lsplit[p, 3c+t], c = 6j+r, t = (hi, hi, lo)
                Lsplit = small.tile([128, 3 * 6 * NCHUNK], f32r, tag="Lsp")
                Lspr = Lsplit.rearrange("p (c t) -> p c t", t=3)
                nc.vector.tensor_copy(out=Lspr[:, :, 0], in_=L6)
                nc.vector.tensor_copy(out=Lspr[:, :, 1], in_=L6)
                lo = small.tile([128, 6 * NCHUNK], f32, tag="lo")
                nc.vector.tensor_tensor(
                    out=lo, in0=L6, in1=Lspr[:, :, 0].bitcast(f32),
                    op=Alu.subtract,
                )
                nc.vector.tensor_copy(out=Lspr[:, :, 2], in_=lo)

                # ---- per chunk: transpose -> K=18 lhsT; d2 matmul; exp
                wxy_all = big.tile([128, NCHUNK, 2 * W], f32r, tag="wxy")
                lhs18 = small.tile([18, NCHUNK, 128], f32r, tag="lhs18")
                for j in range(NCHUNK):
                    tr = ps_tr.tile([18, 128], f32r, tag="tr")
                    nc.tensor.transpose(
                        tr, Lsplit[:, 18 * j : 18 * (j + 1)], ident
                    )
                    nc.vector.tensor_copy(out=lhs18[:, j, :], in_=tr)
                for j in range(NCHUNK):
                    z = ps_z.tile([128, 2 * W], f32, tag="z")
                    nc.tensor.matmul(
                        z, lhs18[:, j, :], grid18, start=True, stop=True
                    )
                    nc.scalar.activation(
                        out=wxy_all[0:KC, j, :], in_=z[0:KC, :], func=Act.Exp,
                        scale=an[0:KC, j : j + 1],
                    )

                # ---- T[p, j, c*W + w] = wx[p,j,w] * col[p, 3j+c]
                T_all = big.tile([128, NCHUNK, CH * W], f32r, tag="T")
                for j in range(NCHUNK):
                    for c in range(CH):
                        nc.vector.tensor_scalar(
                            out=T_all[0:KC, j, c * W : (c + 1) * W],
                            in0=wxy_all[0:KC, j, 0:W].bitcast(f32),
                            scalar1=col9[0:KC, CH * j + c : CH * j + c + 1],
                            scalar2=None,
                            op0=Alu.mult,
                        )

                # ---- main matmuls: out[h, cw] = sum_j wy_j[:, h].T @ T_j
                for i in range(2):
                    phs = []
                    for half in range(2):
                        po = ps_out.tile([HC, NH], f32, tag="out")
                        for j in range(NCHUNK):
                            nc.tensor.matmul(
                                po,
                                wxy_all[0:KC, j, W + HC * i : W + HC * (i + 1)],
                                T_all[0:KC, j, NH * half : NH * (half + 1)],
                                start=(j == 0), stop=(j == NCHUNK - 1),
                            )
                        phs.append(po)

                    # ---- clip to <=1, interleave (w,c) into image row layout
                    osb = outp.tile([HC, W * CH], f32, tag="osb")
                    osr = osb.rearrange("p (w c) -> p w c", c=CH)
                    nc.vector.tensor_scalar(
                        out=osr[:, 0:W, 0], in0=phs[0][:, 0:W],
                        scalar1=1.0, scalar2=None, op0=Alu.min,
                    )
                    nc.vector.tensor_scalar(
                        out=osr[:, 0:HC, 1], in0=phs[0][:, W:NH],
                        scalar1=1.0, scalar2=None, op0=Alu.min,
                    )
                    nc.vector.tensor_scalar(
                        out=osr[:, HC:W, 1], in0=phs[1][:, 0 : W - HC],
                        scalar1=1.0, scalar2=None, op0=Alu.min,
                    )
                    nc.vector.tensor_scalar(
                        out=osr[:, 0:W, 2], in0=phs[1][:, W - HC : NH],
                        scalar1=1.0, scalar2=None, op0=Alu.min,
                    )
                    nc.sync.dma_start(
                        out=img_d[b, HC * i : HC * (i + 1)].rearrange(
                            "h w c -> h (w c)"
                        ),
                        in_=osb,
                    )
    nc.compile()
    return nc


_CACHED = {}


def _get_bass():
    if "nc" not in _CACHED:
        _CACHED["nc"] = build_bass()
    return _CACHED["nc"]


def _const_feed():
    return {
        "grid18": _np_grid18(),
        "ident": np.eye(128, dtype=np.float32),
    }


LAST_RESULT = None


def kernel(positions, colors, sizes, trace=False):
    from concourse.bass_utils import run_bass_kernel_spmd

    global LAST_RESULT
    positions = np.ascontiguousarray(np.asarray(positions, dtype=np.float32))
    colors = np.ascontiguousarray(np.asarray(colors, dtype=np.float32))
    sizes = np.ascontiguousarray(np.asarray(sizes, dtype=np.float32))

    consts = _const_feed()
    nc = _get_bass()
    in_maps = []
    for c in range(NCORES):
        sl = slice(c * BPC, (c + 1) * BPC)
        in_maps.append(
            {
                "positions": positions[sl],
                "colors": colors[sl],
                "sizes": sizes[sl],
                **consts,
            }
        )

    res = run_bass_kernel_spmd(
        nc, in_maps, core_ids=list(range(NCORES)), trace=trace
    )
    LAST_RESULT = res
    return np.concatenate([r["image"] for r in res.results], axis=0)


def _exec_fn(nc):
    """Build a reusable jitted 8-core executor (no donation; kernel writes
    every output element so uninit result buffers are fine)."""
    import jax
    from jax.experimental.shard_map import shard_map
    from jax.sharding import Mesh, PartitionSpec
    from concourse import bass2jax, mybir

    bass2jax.install_neuronx_cc_hook()

    in_names, out_names, out_avals = [], [], []
    for alloc in nc.m.functions[0].allocations:
        if not isinstance(alloc, mybir.MemoryLocationSet):
            continue
        name = alloc.memorylocations[0].name
        if alloc.kind == "ExternalInput":
            in_names.append(name)
        elif alloc.kind == "ExternalOutput":
            out_names.append(name)
            out_avals.append(
                jax.core.ShapedArray(
                    tuple(alloc.tensor_shape), mybir.dt.np(alloc.dtype)
                )
            )
    all_in = in_names + out_names

    def _body(*args):
        outs = bass2jax._bass_exec_p.bind(
            *args,
            out_avals=tuple(out_avals),
            in_names=tuple(all_in),
            out_names=tuple(out_names),
            lowering_input_output_aliases=(),
            sim_require_finite=True,
            sim_require_nnan=True,
            nc=nc,
        )
        return tuple(outs)

    devices = jax.devices()[:NCORES]
    mesh = Mesh(np.asarray(devices), ("core",))
    n_args = len(all_in)
    sharded = jax.jit(
        shard_map(
            _body,
            mesh=mesh,
            in_specs=(PartitionSpec("core"),) * n_args,
            out_specs=(PartitionSpec("core"),) * len(out_names),
            check_rep=False,
        ),
        keep_unused=True,
    )
    return sharded, mesh, in_names, out_names, out_avals


def bench(positions, colors, sizes, iters=50):
    """Steady-state per-execution wall time (s) over 8 cores + output."""
    import time as _time
    import jax
    from jax.sharding import NamedSharding, PartitionSpec

    positions = np.ascontiguousarray(np.asarray(positions, dtype=np.float32))
    colors = np.ascontiguousarray(np.asarray(colors, dtype=np.float32))
    sizes = np.ascontiguousarray(np.asarray(sizes, dtype=np.float32))
    nc = _get_bass()
    sharded, mesh, in_names, out_names, out_avals = _exec_fn(nc)

    consts = _const_feed()
    feed = {
        "positions": positions,
        "colors": colors,
        "sizes": sizes,
        "grid18": np.concatenate([consts["grid18"]] * NCORES, axis=0),
        "ident": np.concatenate([consts["ident"]] * NCORES, axis=0),
    }
    args = [feed[n] for n in in_names]
    args += [
        np.zeros((NCORES * a.shape[0], *a.shape[1:]), a.dtype) for a in out_avals
    ]
    sh = NamedSharding(mesh, PartitionSpec("core"))
    dargs = [jax.device_put(a, sh) for a in args]

    out = sharded(*dargs)
    jax.block_until_ready(out)
    img0 = np.asarray(out[0]).reshape(NCORES, BPC, H, W, CH).reshape(B, H, W, CH)

    times = []
    for _ in range(3):
        t0 = _time.perf_counter()
        for _ in range(iters):
            out = sharded(*dargs)
        jax.block_until_ready(out)
        times.append((_time.perf_counter() - t0) / iters)
    return min(times), img0


# revision 22
# speedup vs baseline: 1.2341x; 1.2341x over previous
"""Trainium2 Bass kernel: differentiable Gaussian-splat renderer.

Math: image[b,h,w,c] = clip( sum_n exp(-a_n*((gx_w-px_n)^2+(gy_h-py_n)^2)) * col[n,c], 0, 1 )
with a_n = 1/(2*sigma_n^2+1e-8), sigma_n = sizes_n*2/H.

The Gaussian separates: exp(-a*(dx^2+dy^2)) = exp(-a*dx^2)*exp(-a*dy^2), so per
frame the image is a matmul over splats:
    image[h, (w,c)] = sum_n wy[n,h] * (wx[n,w]*col[n,c])

d2 is produced by a tiny K-dim polynomial matmul on the PE:
    d2[n, g] = 1*g^2 + (-2p_n)*g + p_n^2
then ONE Exp activation with per-partition scale (-a_n) gives wx|wy.

fp32r (11-bit mantissa) would destroy the d2 cancellation for small sigma, so
both polynomial operands are split hi/lo into fp32r pairs (error-free products,
~2^-22 effective precision) -- K grows 6->18 which costs nothing on the PE.
Chunks are packed at 32-partition offsets so one PE transpose serves all three.

Sharding: data-parallel over B: 16 frames -> 8 cores x 2 frames.
"""

import numpy as np

H = 224
W = 224
NPTS = 381
CH = 3
B = 16
NCORES = 8
BPC = B // NCORES   # frames per core
NCHUNK = 3          # point chunks; n = 3*p + j  (381 = 127*3)
KC = NPTS // NCHUNK  # 127 points per chunk (contraction partitions)
HC = 112            # h-chunk size (224 = 2*112 output partitions)
NH = 336            # moving-dim half (672 = 2*336); >=256 keeps f32r at full rate

PS_TR_BUFS = 1
PS_Z_BUFS = 2
PS_OUT_BUFS = 2


def _round_f32r(x):
    """Round float32 array to fp32r (keep top 11 mantissa bits, round-nearest)."""
    u = np.ascontiguousarray(x, dtype=np.float32).view(np.uint32)
    low = u & 0xFFF
    up = (low > 0x800) | ((low == 0x800) & (((u >> 12) & 1) == 1))
    r = (u & ~np.uint32(0xFFF)) + np.where(up, np.uint32(0x1000), np.uint32(0))
    return r.view(np.float32)


def _np_consts():
    return _np_grid96()


def _np_ident():
    return np.eye(128, dtype=np.float32)


def _np_grid96():
    """[96, 448] fp32r rows; chunk j occupies rows [32j, 32j+18).
    Within a chunk, row 3r+t pairs with lhsT col t in {hi: R_hi, hi: R_lo, lo: R_hi}.
    r in 0..5 = (g^2, g, 1) for x-half cols [0:224], same for y-half [224:448]."""
    g = -1.0 + (2.0 / (W - 1)) * np.arange(W, dtype=np.float64)
    R = np.zeros((6, 2 * W), dtype=np.float64)
    R[0, 0:W] = g * g
    R[1, 0:W] = g
    R[2, 0:W] = 1.0
    R[3, W:] = g * g
    R[4, W:] = g
    R[5, W:] = 1.0
    g18 = np.zeros((18, 2 * W), dtype=np.float32)
    for r in range(6):
        hi = _round_f32r(R[r].astype(np.float32))
        lo = _round_f32r((R[r] - hi.astype(np.float64)).astype(np.float32))
        g18[3 * r + 0] = hi   # pairs L_hi
        g18[3 * r + 1] = lo   # pairs L_hi
        g18[3 * r + 2] = hi   # pairs L_lo
    out = np.zeros((96, 2 * W), dtype=np.float32)
    for j in range(NCHUNK):
        out[32 * j : 32 * j + 18] = g18
    return out


def build_bass():
    import concourse.bacc as bacc
    import concourse.tile as tile
    from concourse import mybir

    f32 = mybir.dt.float32
    f32r = mybir.dt.float32r
    Act = mybir.ActivationFunctionType
    Alu = mybir.AluOpType

    nc = bacc.Bacc("TRN2", debug=False, enable_partition_id=False)

    pk_d = nc.dram_tensor("packed", [BPC, NPTS, 6], f32, kind="ExternalInput")
    cst_d = nc.dram_tensor("consts", [96, 2 * W], f32r, kind="ExternalInput")
    id_d = nc.dram_tensor("ident", [128, 128], f32r, kind="ExternalInput")
    img_d = nc.dram_tensor("image", [BPC, H, W, CH], f32, kind="ExternalOutput")

    with tile.TileContext(nc) as tc:
        with (
            tc.tile_pool(name="const", bufs=1) as constp,
            tc.tile_pool(name="inp", bufs=1) as inp,
            tc.tile_pool(name="small", bufs=2) as small,
            tc.tile_pool(name="big", bufs=2) as big,
            tc.tile_pool(name="outp", bufs=4) as outp,
            tc.tile_pool(name="ps_tr", bufs=PS_TR_BUFS, space="PSUM") as ps_tr,
            tc.tile_pool(name="ps_z", bufs=PS_Z_BUFS, space="PSUM") as ps_z,
            tc.tile_pool(name="ps_out", bufs=PS_OUT_BUFS, space="PSUM") as ps_out,
            tc.tile_pool(name="ps_w", bufs=1, space="PSUM") as ps_w,
        ):
            # ---- grid const first (gates the z matmuls), packed input second;
            # interleaved chunking: point n = 3p+j -> (partition p, chunk j)
            pk = inp.tile([128, BPC, 6 * NCHUNK], f32)  # cols (b, (j, k))
            nc.sync.dma_start(
                out=pk[0:KC],
                in_=pk_d[:].rearrange("b (p j) k -> p b (j k)", j=NCHUNK),
            )
            grid96 = constp.tile([96, 2 * W], f32r)
            nc.sync.dma_start(out=grid96, in_=cst_d[:])
            ident = constp.tile([128, 128], f32r)
            nc.sync.dma_start(out=ident, in_=id_d[:])
            # f32 zero source for initializing f32r pad columns (f32r memset
            # is invalid ISA; a rounding tensor_copy is the legal producer)
            zpad = constp.tile([128, 14 * NCHUNK], f32)
            nc.vector.memset(zpad, 0.0)

            # ---- PE warmup: dummy matmuls so the HAM clock-gate opens
            # before the real pipeline reaches the PE
            wsb = constp.tile([1, 128], f32)
            nc.gpsimd.memset(wsb, 0.0)
            wps = ps_w.tile([128, 128], f32, tag="w")
            for _ in range(4):
                nc.tensor.matmul(wps, wsb, wsb, start=True, stop=True)

            # ---- both frames' param chains batched into single wide ops
            # an = -1/(2*sigma^2 + 1e-8)  [*, b, j]
            szr = pk.rearrange("p b (j k) -> p b j k", k=6)[:, :, :, 5]
            sq = small.tile([128, BPC, NCHUNK], f32, tag="sq")
            nc.scalar.activation(
                out=sq[0:KC], in_=szr[0:KC], func=Act.Square, scale=2.0 / H
            )
            u = small.tile([128, BPC, NCHUNK], f32, tag="u")
            nc.vector.tensor_scalar(
                out=u[0:KC], in0=sq[0:KC], scalar1=-2.0, scalar2=-1e-8,
                op0=Alu.mult, op1=Alu.add,
            )
            an2 = small.tile([128, BPC, NCHUNK], f32, tag="an")
            nc.vector.reciprocal(out=an2[0:KC], in_=u[0:KC])

            # ---- polynomial coeffs L6[p, b, 6j+r] = (1,-2px,px^2, 1,-2py,py^2)
            pkr = pk.rearrange("p b (j k) -> p b j k", k=6)
            posr = pkr[:, :, :, 0:2]   # [*, b, j, e]
            L6 = small.tile([128, BPC, 6 * NCHUNK], f32, tag="L6")
            nc.vector.memset(L6, 0.0)
            axr = L6.rearrange("p b (j s r) -> p b j s r", s=2, r=3)[0:KC]
            nc.vector.memset(axr[:, :, :, :, 0], 1.0)
            nc.vector.tensor_scalar(
                out=axr[:, :, :, :, 1], in0=posr[0:KC], scalar1=-2.0,
                scalar2=None, op0=Alu.mult,
            )
            nc.vector.tensor_mul(axr[:, :, :, :, 2], posr[0:KC], posr[0:KC])

            # ---- fp32r hi/lo split, chunks packed at 32-col offsets:
            # Lsplit[p, b, 32j+3r+t], t = (hi, hi, lo); cols 18..31 of each
            # block pair with zero grid rows so their content is irrelevant.
            Lsplit2 = small.tile([128, BPC, 96], f32r, tag="Lsp")
            Lspr2 = Lsplit2.rearrange("p b (j c) -> p b j c", j=NCHUNK)
            for bb in range(BPC):
                nc.vector.tensor_copy(
                    out=Lspr2[:, bb, :, 18:32],
                    in_=zpad.rearrange("p (j c) -> p j c", j=NCHUNK),
                )
            Lsp3 = Lspr2[
                :, :, :, 0:18
            ].rearrange("p b j (r t) -> p b j r t", t=3)
            L6j = L6.rearrange("p b (j r) -> p b j r", r=6)
            nc.vector.tensor_copy(out=Lsp3[:, :, :, :, 0], in_=L6j)
            nc.vector.tensor_copy(out=Lsp3[:, :, :, :, 1], in_=L6j)
            lo = small.tile([128, BPC, 6 * NCHUNK], f32, tag="lo")
            lor = lo.rearrange("p b (j r) -> p b j r", r=6)
            nc.vector.tensor_tensor(
                out=lor, in0=L6j, in1=Lsp3[:, :, :, :, 0].bitcast(f32),
                op=Alu.subtract,
            )
            nc.vector.tensor_copy(out=Lsp3[:, :, :, :, 2], in_=lor)

            for b in range(BPC):
                pkb = pkr[:, b]         # [*, j, k]
                colr = pkb[:, :, 2:5]   # [*, j, c]
                an = an2[:, b]
                Lsplit = Lsplit2[:, b]

                # ---- one transpose for all chunks -> lhs96 [96, 128]
                tr = ps_tr.tile([96, 128], f32r, tag="tr")
                nc.tensor.transpose(tr, Lsplit, ident)
                lhs96 = small.tile([96, 128], f32r, tag="lhs96")
                nc.vector.tensor_copy(out=lhs96, in_=tr)

                # ---- per chunk: d2 matmul (K=18 at partition 32j); exp
                wxy_all = big.tile([128, NCHUNK, 2 * W], f32r, tag="wxy")
                for j in range(NCHUNK):
                    z = ps_z.tile([128, 2 * W], f32, tag="z")
                    nc.tensor.matmul(
                        z,
                        lhs96[32 * j : 32 * j + 18, :],
                        grid96[32 * j : 32 * j + 18, :],
                        start=True, stop=True,
                    )
                    nc.scalar.activation(
                        out=wxy_all[0:KC, j, :], in_=z[0:KC, :], func=Act.Exp,
                        scale=an[0:KC, j : j + 1],
                    )

                # ---- T[p, j, 3w+c] = wx[p,j,w] * col[p,(j,c)]  (w-major interleave)
                T_all = big.tile([128, NCHUNK, CH * W], f32r, tag="T")
                Twc = T_all.rearrange("p j (w c) -> p j w c", c=CH)
                for j in range(NCHUNK):
                    for c in range(CH):
                        eng = nc.gpsimd if c == 2 else nc.vector
                        eng.tensor_scalar(
                            out=Twc[0:KC, j, :, c],
                            in0=wxy_all[0:KC, j, 0:W].bitcast(f32),
                            scalar1=colr[0:KC, j, c : c + 1],
                            scalar2=None,
                            op0=Alu.mult,
                        )

                # ---- main matmuls into per-h-chunk 2-bank psum tiles;
                # bank x = half holds s' = 3w+c range [336x, 336x+336)
                img_r = img_d[b].rearrange("(i p) w c -> p i (w c)", i=2)
                for i in range(2):
                    po = ps_out.tile([HC, 2, 512], f32, tag="out")
                    for half in range(2):
                        for j in range(NCHUNK):
                            nc.tensor.matmul(
                                po[:, half, 0:NH],
                                wxy_all[0:KC, j, W + HC * i : W + HC * (i + 1)],
                                T_all[0:KC, j, NH * half : NH * (half + 1)],
                                start=(j == 0), stop=(j == NCHUNK - 1),
                            )
                    osb = outp.tile([HC, W * CH], f32, tag="osb")
                    nc.vector.tensor_scalar(
                        out=osb.rearrange("p (x s) -> p x s", s=NH),
                        in0=po[:, :, 0:NH],
                        scalar1=1.0, scalar2=None, op0=Alu.min,
                    )
                    dma_eng = nc.scalar if b == BPC - 1 else nc.sync
                    dma_eng.dma_start(out=img_r[:, i, :], in_=osb)
    nc.compile()
    return nc


_CACHED = {}


def _get_bass():
    if "nc" not in _CACHED:
        _CACHED["nc"] = build_bass()
    return _CACHED["nc"]


def _pack_inputs(positions, colors, sizes):
    pk = np.empty((positions.shape[0], NPTS, 6), dtype=np.float32)
    pk[:, :, 0:2] = positions
    pk[:, :, 2:5] = colors
    pk[:, :, 5] = sizes
    return pk


LAST_RESULT = None


def kernel(positions, colors, sizes, trace=False):
    from concourse.bass_utils import run_bass_kernel_spmd

    global LAST_RESULT
    positions = np.ascontiguousarray(np.asarray(positions, dtype=np.float32))
    colors = np.ascontiguousarray(np.asarray(colors, dtype=np.float32))
    sizes = np.ascontiguousarray(np.asarray(sizes, dtype=np.float32))

    pk = _pack_inputs(positions, colors, sizes)
    cst = _np_consts()
    nc = _get_bass()
    in_maps = []
    for c in range(NCORES):
        sl = slice(c * BPC, (c + 1) * BPC)
        in_maps.append({"packed": pk[sl], "consts": cst, "ident": _np_ident()})

    res = run_bass_kernel_spmd(
        nc, in_maps, core_ids=list(range(NCORES)), trace=trace
    )
    LAST_RESULT = res
    return np.concatenate([r["image"] for r in res.results], axis=0)


def _exec_fn(nc):
    """Build a reusable jitted 8-core executor (no donation; kernel writes
    every output element so uninit result buffers are fine)."""
    import jax
    from jax.experimental.shard_map import shard_map
    from jax.sharding import Mesh, PartitionSpec
    from concourse import bass2jax, mybir

    bass2jax.install_neuronx_cc_hook()

    in_names, out_names, out_avals = [], [], []
    for alloc in nc.m.functions[0].allocations:
        if not isinstance(alloc, mybir.MemoryLocationSet):
            continue
        name = alloc.memorylocations[0].name
        if alloc.kind == "ExternalInput":
            in_names.append(name)
        elif alloc.kind == "ExternalOutput":
            out_names.append(name)
            out_avals.append(
                jax.core.ShapedArray(
                    tuple(alloc.tensor_shape), mybir.dt.np(alloc.dtype)
                )
            )
    all_in = in_names + out_names

    def _body(*args):
        outs = bass2jax._bass_exec_p.bind(
            *args,
            out_avals=tuple(out_avals),
            in_names=tuple(all_in),
            out_names=tuple(out_names),
            lowering_input_output_aliases=(),
            sim_require_finite=True,
            sim_require_nnan=True,
            nc=nc,
        )
        return tuple(outs)

    devices = jax.devices()[:NCORES]
    mesh = Mesh(np.asarray(devices), ("core",))
    n_args = len(all_in)
    sharded = jax.jit(
        shard_map(
            _body,
            mesh=mesh,
            in_specs=(PartitionSpec("core"),) * n_args,
            out_specs=(PartitionSpec("core"),) * len(out_names),
            check_rep=False,
        ),
        keep_unused=True,
    )
    return sharded, mesh, in_names, out_names, out_avals


def bench(positions, colors, sizes, iters=50):
    """Steady-state per-execution wall time (s) over 8 cores + output."""
    import time as _time
    import jax
    from jax.sharding import NamedSharding, PartitionSpec

    positions = np.ascontiguousarray(np.asarray(positions, dtype=np.float32))
    colors = np.ascontiguousarray(np.asarray(colors, dtype=np.float32))
    sizes = np.ascontiguousarray(np.asarray(sizes, dtype=np.float32))
    nc = _get_bass()
    sharded, mesh, in_names, out_names, out_avals = _exec_fn(nc)

    feed = {
        "packed": _pack_inputs(positions, colors, sizes),
        "consts": np.concatenate([_np_consts()] * NCORES, axis=0),
        "ident": np.concatenate([_np_ident()] * NCORES, axis=0),
    }
    args = [feed[n] for n in in_names]
    args += [
        np.zeros((NCORES * a.shape[0], *a.shape[1:]), a.dtype) for a in out_avals
    ]
    sh = NamedSharding(mesh, PartitionSpec("core"))
    dargs = [jax.device_put(a, sh) for a in args]

    out = sharded(*dargs)
    jax.block_until_ready(out)
    img0 = np.asarray(out[0]).reshape(NCORES, BPC, H, W, CH).reshape(B, H, W, CH)

    times = []
    for _ in range(3):
        t0 = _time.perf_counter()
        for _ in range(iters):
            out = sharded(*dargs)
        jax.block_until_ready(out)
        times.append((_time.perf_counter() - t0) / iters)
    return min(times), img0
